# revision 21
# baseline (speedup 1.0000x reference)
"""DenseCRF loss kernel for Trainium2, data-parallel over batch on 8 NeuronCores.

reference:
  seg = bilinear_resize(segmentations, 128->64)            # [N,K,64,64]
  f_i = [x_i/50, y_i/50, r_i/15, g_i/15, b_i/15]           # 5-dim bilateral feature
  W_ij = exp(-0.5*|f_i - f_j|^2)                           # [P,P], P=4096
  loss = WEIGHT * (-sum_k s_k^T W s_k) / N

Per core (1 image). W is symmetric: only the lower triangle at 128x128 block
granularity is computed. G(i,j) = f_i.f_j - q_i - q_j (q = 0.5|f|^2) is one
28-row bf16 matmul per block; features are split hi/lo so products are exact
in fp32 PSUM. FA/FB [28,P] are STAGED ON HOST (pure input packing: scaling,
bf16 hi/lo split, row duplication) and DMA'd in; segmentations are host-
rearranged to [y, k*x] bf16. The seg resize itself runs on-device (PE).

G blocks stream through two [128,1536] PSUM buffers; ScalarE Exp's each
batch (46 instrs total, no bias). The x2 for sub-diagonal blocks rides a
2x-scaled copy of S^T (STt2) used by the acc matmuls; true-diagonal 128-blocks
use the 1x copy (one PSUM accumulation start/stop per group bank). Per column
group g an acc PSUM tile accumulates S^T E over all row chunks; a DVE
mul+reduce against Srow forms partials[:, g] (tensor_tensor_reduce faults the
exec unit on hw — do not use). partials [K,NG] DMA out; host sums and scales.
"""

import sys

sys.path.insert(0, "/opt/trn_rl_repo")

import numpy as np
import ml_dtypes

import concourse.bass as bass
import concourse.tile as tile
from concourse import bacc, bass_isa, mybir
from concourse.bass_utils import run_bass_kernel_spmd

F32 = mybir.dt.float32
BF16 = mybir.dt.bfloat16
AF = mybir.ActivationFunctionType
ALU = mybir.AluOpType
BF = ml_dtypes.bfloat16

N, C, K = 8, 3, 21
H, W = 64, 64
P = H * W  # 4096
SIGMA_RGB = 15.0
SXY = 100.0 * 0.5  # sigma_xy * scale
WEIGHT = 1e-8
NB = 32  # 128-row chunks of P
NG = 8  # 512-col groups of P
FR = 28  # feature rows
BW = 1536  # exp batch width (3 PSUM banks)
FINE_DIAG = True  # 128-granular triangle on the diagonal 512-blocks
USE_TTR = False  # fused DVE tensor_tensor_reduce for the dots


def _resize_matrix():
    """[64,128] weights of jax.image.resize(..., method='bilinear') along one dim
    (triangle kernel, antialias=True, scale=0.5, renormalized)."""
    y = np.arange(128, dtype=np.float64)[:, None]
    sample = 2.0 * np.arange(64, dtype=np.float64)[None, :] + 0.5
    w = np.maximum(0.0, 1.0 - 0.5 * np.abs(y - sample))
    w = w / w.sum(axis=0, keepdims=True)
    return np.ascontiguousarray(w.T.astype(np.float32))  # [64,128]


def _consts():
    rt = np.ascontiguousarray(_resize_matrix().T).astype(BF)  # [128,64]
    idf = np.eye(K, dtype=np.float32)
    return dict(rt=rt, idf=idf)


def _pos_rows():
    """Constant position rows 16..27 of FA/FB (bf16 hi/lo, exact-q)."""
    i = np.arange(P, dtype=np.float32)
    px = (i % 64).astype(np.float32) / np.float32(SXY)
    py = (i // 64).astype(np.float32) / np.float32(SXY)
    pos = np.stack([px, py])  # [2,P] f32
    ph = pos.astype(BF)
    pl = (pos - ph.astype(np.float32)).astype(BF)
    pf = ph.astype(np.float64) + pl.astype(np.float64)
    qpos = -0.5 * (pf[0] ** 2 + pf[1] ** 2)  # [P] f64
    qph = qpos.astype(np.float32).astype(BF)
    qpl = (qpos - qph.astype(np.float64)).astype(np.float32).astype(BF)
    one = np.ones(P, dtype=BF)
    A = np.empty((12, P), dtype=BF)
    B = np.empty((12, P), dtype=BF)
    A[0:2], A[2:4], A[4:6], A[6:8] = ph, ph, pl, pl
    B[0:2], B[2:4], B[4:6], B[6:8] = ph, pl, ph, pl
    A[8], A[9], A[10], A[11] = qph, qpl, one, one
    B[8], B[9], B[10], B[11] = one, one, qph, qpl
    return A, B


_POSA, _POSB = _pos_rows()


def _features(img):
    """FA/FB [28,P] bf16 for one image [C,H,W] f32 (hi/lo exact split)."""
    inv15 = np.float32(1.0) / np.float32(SIGMA_RGB)
    c = img.reshape(C, P).astype(np.float32) * inv15
    fh = c.astype(BF)
    fl = (c - fh.astype(np.float32)).astype(BF)
    q3 = (c * c).sum(axis=0, dtype=np.float32)
    qn = np.float32(-0.5) * q3
    qch = qn.astype(BF)
    qcl = (qn - qch.astype(np.float32)).astype(BF)
    one = np.ones(P, dtype=BF)
    FA = np.empty((FR, P), dtype=BF)
    FB = np.empty((FR, P), dtype=BF)
    FA[0:3], FA[3:6], FA[6:9], FA[9:12] = fh, fh, fl, fl
    FB[0:3], FB[3:6], FB[6:9], FB[9:12] = fh, fl, fh, fl
    FA[12], FA[13], FA[14], FA[15] = qch, qcl, one, one
    FB[12], FB[13], FB[14], FB[15] = one, one, qch, qcl
    FA[16:28] = _POSA
    FB[16:28] = _POSB
    return FA, FB


def _prep(images, segmentations):
    """Per-core input dicts from full [N,...] inputs."""
    images = np.asarray(images, dtype=np.float32)
    segmentations = np.asarray(segmentations, dtype=np.float32)
    consts = _consts()
    maps = []
    for n in range(N):
        FA, FB = _features(images[n])
        segy = np.ascontiguousarray(
            segmentations[n].transpose(1, 0, 2).reshape(128, K * 128)
        ).astype(BF)  # [y, (k,x)]
        maps.append(dict(fa=FA, fb=FB, seg=segy, **consts))
    return maps


def _batches():
    """Work-item schedule: list of batches; each batch is a list of items
    (g, b, width, off) sharing one [128,BW] PSUM tile / one Exp. Diagonal
    512-blocks are emitted at 128 granularity packed [512,384,128,256]."""
    batches = []
    cur, off = [], 0
    for g in range(NG - 1, -1, -1):
        if FINE_DIAG:
            # diag batch: chunks 4g+3 (w512), 4g+2 (384), 4g (128), 4g+1 (256)
            batches.append([
                (g, 4 * g + 3, 512, 0),
                (g, 4 * g + 2, 384, 512),
                (g, 4 * g + 0, 128, 896),
                (g, 4 * g + 1, 256, 1024),
            ])
            lastb = 4 * g + 3
        else:
            lastb = 4 * g - 1
        for b in range(NB - 1, lastb, -1):  # full-width chunks, descending
            cur.append((g, b, 512, off))
            off += 512
            if off == BW:
                batches.append(cur)
                cur, off = [], 0
    if cur:
        batches.append(cur)
    return batches


def _acc_writers(items):
    """Per group: ordered acc-matmul descriptors (g, b, scaled, lo, hi) in
    emission order, with start/stop flags per 128-col region computed by a
    forward (fresh) and backward (last-writer) pass."""
    per_g = {g: [] for g in range(NG)}
    for g, b, w, off in items:
        i = b - 4 * g
        if 0 <= i < 4 and FINE_DIAG:  # diag chunk: sub-diag (x2) + diag blk (x1)
            if i > 0:
                per_g[g].append([g, b, True, 0, i * 128, off])
            per_g[g].append([g, b, False, i * 128, (i + 1) * 128, off])
        elif 0 <= i < 4:  # coarse: whole diag chunk at x1
            per_g[g].append([g, b, False, 0, 512, off])
        else:
            per_g[g].append([g, b, True, 0, 512, off])
    # PSUM accumulation groups are per 2KB bank: start=True marks the WHOLE
    # bank pending-zero (first touch of each byte overwrites, later touches
    # accumulate). So: exactly one start (first matmul of the group's bank)
    # and one stop (last matmul).
    flags = {}
    for g, lst in per_g.items():
        n = len(lst)
        flags[g] = [
            (tuple(d), i == 0, i == n - 1) for i, d in enumerate(lst)
        ]
    return flags


def _build():
    nc = bacc.Bacc()
    fa_d = nc.dram_tensor("fa", [FR, P], BF16, kind="ExternalInput")
    fb_d = nc.dram_tensor("fb", [FR, P], BF16, kind="ExternalInput")
    seg_d = nc.dram_tensor("seg", [128, K * 128], BF16, kind="ExternalInput")
    rt_d = nc.dram_tensor("rt", [128, 64], BF16, kind="ExternalInput")
    idf_d = nc.dram_tensor("idf", [K, K], F32, kind="ExternalInput")
    out_d = nc.dram_tensor("out", [K, NG], F32, kind="ExternalOutput")

    batches = _batches()
    nbatch = len(batches)
    # map (g,b) -> batch index for acc scheduling; group -> last batch idx
    item_batch = {}
    glast = {}
    for bi, items in enumerate(batches):
        for g, b, w, off in items:
            item_batch[(g, b)] = bi
            glast[g] = bi
    acc_flags = _acc_writers([it for bt in batches for it in bt])
    # per-batch acc descriptors (in group emission order)
    bat_accs = [[] for _ in range(nbatch)]
    for g in range(NG - 1, -1, -1):
        for (gg, b, scaled, lo, hi, off), st, sp in acc_flags[g]:
            bat_accs[item_batch[(gg, b)]].append((gg, b, scaled, lo, hi, off, st, sp))

    with tile.TileContext(nc) as tc:
        with (
            tc.tile_pool(name="pp", bufs=1) as pp,
            tc.tile_pool(name="gq", bufs=2, space="PSUM") as gq,
            tc.tile_pool(name="aq", bufs=2, space="PSUM") as aq,
            tc.tile_pool(name="ep", bufs=9) as ep,
            tc.tile_pool(name="dp", bufs=2) as dp,
        ):
            FAs = pp.tile([FR, P], BF16, tag="fa", name="FAs")
            FBs = pp.tile([FR, P], BF16, tag="fb", name="FBs")
            seg_s = pp.tile([128, K * 128], BF16, tag="seg")
            rt_s = pp.tile([128, 64], BF16, tag="rt")
            idf_s = pp.tile([K, K], F32, tag="idf")
            At = pp.tile([128, K * 64], BF16, tag="At")
            Srow = [pp.tile([K, 512], F32, tag=f"sr{y}", name=f"sr{y}") for y in range(NG)]
            STt1 = [pp.tile([128, 8 * K], BF16, tag=f"t1{i}", name=f"t1{i}") for i in range(4)]
            STt2 = [pp.tile([128, 8 * K], BF16, tag=f"t2{i}", name=f"t2{i}") for i in range(4)]
            partials = pp.tile([K, NG], F32, tag="partials")

            qS = nc.sync.dma_start
            qP = nc.gpsimd.dma_start

            # ---- input loads. FA/FB head slices land first (smallest head =
            # batch 0's columns) so the G pipeline starts ~2us in; the rest
            # follows behind the seg halves. ----
            H1, H2 = 3584, 2560
            qS(FAs[:, H1:], fa_d[:, H1:])
            qP(FBs[:, H1:], fb_d[:, H1:])
            qS(FAs[:, H2:H1], fa_d[:, H2:H1])
            qP(FBs[:, H2:H1], fb_d[:, H2:H1])
            qS(rt_s[:], rt_d[:])
            qP(seg_s[:, : 8 * 128], seg_d[:, : 8 * 128])
            qS(seg_s[:, 8 * 128 : 16 * 128], seg_d[:, 8 * 128 : 16 * 128])
            qP(idf_s[:], idf_d[:])
            qS(seg_s[:, 16 * 128 :], seg_d[:, 16 * 128 :])
            qP(FBs[:, :H2], fb_d[:, :H2])
            qS(FAs[:, :H2], fa_d[:, :H2])

            # ---- PE p-state warmup: keep the PE busy before the first G
            # batch so the early matmuls run at mid/full clock ----
            wsb = pp.tile([2, 256], BF16, tag="warm")
            nc.vector.memset(wsb[:], 1.0)
            for wi in range(8):
                wps = aq.tile([128, 512], F32, tag="a", name=f"warm{wi}")
                nc.tensor.matmul(
                    wps[0:1, 0:256], wsb[:, 0:1], wsb[:], start=True, stop=True
                )

            at3 = At[:, :].rearrange("x (k y) -> x k y", k=K, y=64)

            def at_block(k0, k1):
                aps = aq.tile([128, 512], F32, tag="a", name=f"at{k0}")
                for k in range(k0, k1):
                    nc.tensor.matmul(
                        aps[:, (k - k0) * 64 : (k - k0 + 1) * 64],
                        seg_s[:, k * 128 : (k + 1) * 128],
                        rt_s[:],
                        start=True, stop=True,
                    )
                nc.vector.tensor_copy(At[:, k0 * 64 : k1 * 64], aps[:, : (k1 - k0) * 64])

            def srow_stage(yb):
                sps = aq.tile([128, 512], F32, tag="a", name=f"sr{yb}")
                for yl in range(8):
                    yp = yb * 8 + yl
                    nc.tensor.matmul(
                        sps[0:K, yl * 64 : (yl + 1) * 64],
                        at3[:, :, yp], rt_s[:],
                        start=True, stop=True,
                    )
                nc.vector.tensor_copy(Srow[yb][:], sps[0:K, :])

            def stt_stage(bi):
                tps = aq.tile([128, 512], F32, tag="a", name=f"st{bi}")
                for j in range(8):
                    b = 8 * bi + j
                    yb, rest = divmod(b * 128, 512)
                    nc.tensor.transpose(
                        tps[:, j * K : (j + 1) * K],
                        Srow[yb][:, rest : rest + 128],
                        idf_s[:],
                    )
                nc.vector.tensor_copy(STt1[bi][:], tps[:, : 8 * K])
                nc.vector.tensor_scalar_mul(STt2[bi][:], tps[:, : 8 * K], 2.0)

            # seg-pipeline emission steps: all scratch-tile (aq) requests must
            # precede the first acc-tile request (batch ACC_LAG) or the
            # 2-buffer rotation deadlocks against a live accumulator.
            seg_steps = {
                1: [lambda: at_block(0, 8)],
                2: [lambda: at_block(8, 16)],
                3: [lambda: at_block(16, 21), lambda: srow_stage(7)],
                4: [lambda: srow_stage(6), lambda: stt_stage(3)],
                5: [lambda: srow_stage(5), lambda: srow_stage(4)],
                6: [lambda: stt_stage(2), lambda: srow_stage(3)],
                7: [lambda: srow_stage(2), lambda: stt_stage(1),
                    lambda: srow_stage(1), lambda: srow_stage(0),
                    lambda: stt_stage(0)],
            }
            ACC_LAG = 8  # batches between exp and its acc matmuls

            acc_tiles = {}

            def emit_accs(bi, ets):
                for g, b, scaled, lo, hi, off, st, sp in bat_accs[bi]:
                    if g not in acc_tiles:
                        acc_tiles[g] = aq.tile([128, 512], F32, tag="a", name=f"acc{g}")
                    stt = (STt2 if scaled else STt1)[b // 8]
                    nc.tensor.matmul(
                        acc_tiles[g][0:K, lo:hi],
                        stt[:, (b % 8) * K : (b % 8 + 1) * K],
                        ets[bi][:, off + lo : off + hi],
                        start=st, stop=sp,
                        skip_group_check=True,
                    )

            def emit_dots(bi):
                done = {g for g in range(NG) if glast[g] == bi}
                for g in sorted(done, reverse=True):
                    dsc = dp.tile([K, 512], BF16, tag="d", name=f"dsc{g}")
                    if USE_TTR:
                        nc.vector.tensor_tensor_reduce(
                            dsc[:], acc_tiles[g][0:K, :], Srow[g][:],
                            1.0, 0.0, ALU.mult, ALU.add,
                            partials[:, g : g + 1],
                        )
                    else:
                        nc.vector.tensor_mul(dsc[:], acc_tiles[g][0:K, :], Srow[g][:])
                        nc.vector.tensor_reduce(
                            partials[:, g : g + 1], dsc[:],
                            mybir.AxisListType.X, ALU.add,
                        )

            ets = {}
            for bi, items in enumerate(batches):
                gt = gq.tile([128, BW], F32, tag="g", name=f"g{bi}")
                width = max(off + w for _, _, w, off in items)
                for g, b, w, off in items:
                    nc.tensor.matmul(
                        gt[:, off : off + w],
                        FAs[:, b * 128 : (b + 1) * 128],
                        FBs[:, 512 * g : 512 * g + w],
                        start=True, stop=True,
                    )
                et = ep.tile([128, BW], BF16, tag="e", name=f"e{bi}")
                ets[bi] = et
                nc.scalar.activation(et[:, :width], gt[:, :width], AF.Exp)
                for fn in seg_steps.get(bi, ()):
                    fn()
                if bi >= ACC_LAG:
                    emit_accs(bi - ACC_LAG, ets)
                    emit_dots(bi - ACC_LAG)
            for bi in range(max(0, nbatch - ACC_LAG), nbatch):
                emit_accs(bi, ets)
                emit_dots(bi)

            # ---- tail: DMA the per-group partials; all summing on host ----
            nc.sync.dma_start(out_d[:], partials[:])

    nc.finalize()
    return nc


_CACHE = {}


def _get_nc():
    if "nc" not in _CACHE:
        _CACHE["nc"] = _build()
    return _CACHE["nc"]


def kernel(images: np.ndarray, segmentations: np.ndarray) -> np.ndarray:
    images = np.ascontiguousarray(np.asarray(images, dtype=np.float32))
    segmentations = np.ascontiguousarray(np.asarray(segmentations, dtype=np.float32))
    assert images.shape == (N, C, H, W) and segmentations.shape == (N, K, 128, 128)
    nc = _get_nc()
    in_maps = _prep(images, segmentations)
    res = run_bass_kernel_spmd(nc, in_maps, list(range(N)))
    total = sum(float(res.results[n]["out"].sum()) for n in range(N))
    return np.array([-WEIGHT / N * total], dtype=np.float32)


if __name__ == "__main__":
    rng = np.random.RandomState(0)
    img = rng.rand(N, C, H, W).astype(np.float32) * 255.0
    seg = rng.rand(N, K, 128, 128).astype(np.float32)
    print(kernel(img, seg))


# revision 24
# speedup vs baseline: 1.0050x; 1.0050x over previous
"""DenseCRF loss kernel for Trainium2, data-parallel over batch on 8 NeuronCores.

reference:
  seg = bilinear_resize(segmentations, 128->64)            # [N,K,64,64]
  f_i = [x_i/50, y_i/50, r_i/15, g_i/15, b_i/15]           # 5-dim bilateral feature
  W_ij = exp(-0.5*|f_i - f_j|^2)                           # [P,P], P=4096
  loss = WEIGHT * (-sum_k s_k^T W s_k) / N

Per core (1 image). W is symmetric: only the lower triangle at 128x128 block
granularity is computed. G(i,j) = f_i.f_j - q_i - q_j (q = 0.5|f|^2) is one
28-row bf16 matmul per block; features are split hi/lo so products are exact
in fp32 PSUM. FA/FB [28,P] are STAGED ON HOST (pure input packing: scaling,
bf16 hi/lo split, row duplication) and DMA'd in; segmentations are host-
rearranged to [y, k*x] bf16. The seg resize itself runs on-device (PE).

G blocks stream through two [128,1536] PSUM buffers; ScalarE Exp's each
batch (46 instrs total, no bias). The x2 for sub-diagonal blocks rides a
2x-scaled copy of S^T (STt2) used by the acc matmuls; true-diagonal 128-blocks
use the 1x copy (one PSUM accumulation start/stop per group bank). Per column
group g an acc PSUM tile accumulates S^T E over all row chunks; a DVE
mul+reduce against Srow forms partials[:, g] (tensor_tensor_reduce faults the
exec unit on hw — do not use). partials [K,NG] DMA out; host sums and scales.
"""

import sys

sys.path.insert(0, "/opt/trn_rl_repo")

import numpy as np
import ml_dtypes

import concourse.bass as bass
import concourse.tile as tile
from concourse import bacc, bass_isa, mybir
from concourse.bass_utils import run_bass_kernel_spmd

F32 = mybir.dt.float32
BF16 = mybir.dt.bfloat16
AF = mybir.ActivationFunctionType
ALU = mybir.AluOpType
BF = ml_dtypes.bfloat16

N, C, K = 8, 3, 21
H, W = 64, 64
P = H * W  # 4096
SIGMA_RGB = 15.0
SXY = 100.0 * 0.5  # sigma_xy * scale
WEIGHT = 1e-8
NB = 32  # 128-row chunks of P
NG = 8  # 512-col groups of P
FR = 28  # feature rows
BW = 1536  # exp batch width (3 PSUM banks)
FINE_DIAG = True  # 128-granular triangle on the diagonal 512-blocks
USE_TTR = False  # fused DVE tensor_tensor_reduce for the dots
DVE_CHUNKS = (31, 30, 29)  # group-0 chunks whose exp runs on DVE, not ScalarE


def _resize_matrix():
    """[64,128] weights of jax.image.resize(..., method='bilinear') along one dim
    (triangle kernel, antialias=True, scale=0.5, renormalized)."""
    y = np.arange(128, dtype=np.float64)[:, None]
    sample = 2.0 * np.arange(64, dtype=np.float64)[None, :] + 0.5
    w = np.maximum(0.0, 1.0 - 0.5 * np.abs(y - sample))
    w = w / w.sum(axis=0, keepdims=True)
    return np.ascontiguousarray(w.T.astype(np.float32))  # [64,128]


def _consts():
    rt = np.ascontiguousarray(_resize_matrix().T).astype(BF)  # [128,64]
    idf = np.eye(K, dtype=np.float32)
    return dict(rt=rt, idf=idf)


def _pos_rows():
    """Constant position rows 16..27 of FA/FB (bf16 hi/lo, exact-q)."""
    i = np.arange(P, dtype=np.float32)
    px = (i % 64).astype(np.float32) / np.float32(SXY)
    py = (i // 64).astype(np.float32) / np.float32(SXY)
    pos = np.stack([px, py])  # [2,P] f32
    ph = pos.astype(BF)
    pl = (pos - ph.astype(np.float32)).astype(BF)
    pf = ph.astype(np.float64) + pl.astype(np.float64)
    qpos = -0.5 * (pf[0] ** 2 + pf[1] ** 2)  # [P] f64
    qph = qpos.astype(np.float32).astype(BF)
    qpl = (qpos - qph.astype(np.float64)).astype(np.float32).astype(BF)
    one = np.ones(P, dtype=BF)
    A = np.empty((12, P), dtype=BF)
    B = np.empty((12, P), dtype=BF)
    A[0:2], A[2:4], A[4:6], A[6:8] = ph, ph, pl, pl
    B[0:2], B[2:4], B[4:6], B[6:8] = ph, pl, ph, pl
    A[8], A[9], A[10], A[11] = qph, qpl, one, one
    B[8], B[9], B[10], B[11] = one, one, qph, qpl
    return A, B


_POSA, _POSB = _pos_rows()


def _features(img):
    """FA/FB [28,P] bf16 for one image [C,H,W] f32 (hi/lo exact split)."""
    inv15 = np.float32(1.0) / np.float32(SIGMA_RGB)
    c = img.reshape(C, P).astype(np.float32) * inv15
    fh = c.astype(BF)
    fl = (c - fh.astype(np.float32)).astype(BF)
    q3 = (c * c).sum(axis=0, dtype=np.float32)
    qn = np.float32(-0.5) * q3
    qch = qn.astype(BF)
    qcl = (qn - qch.astype(np.float32)).astype(BF)
    one = np.ones(P, dtype=BF)
    FA = np.empty((FR, P), dtype=BF)
    FB = np.empty((FR, P), dtype=BF)
    FA[0:3], FA[3:6], FA[6:9], FA[9:12] = fh, fh, fl, fl
    FB[0:3], FB[3:6], FB[6:9], FB[9:12] = fh, fl, fh, fl
    FA[12], FA[13], FA[14], FA[15] = qch, qcl, one, one
    FB[12], FB[13], FB[14], FB[15] = one, one, qch, qcl
    FA[16:28] = _POSA
    FB[16:28] = _POSB
    return FA, FB


def _prep(images, segmentations):
    """Per-core input dicts from full [N,...] inputs."""
    images = np.asarray(images, dtype=np.float32)
    segmentations = np.asarray(segmentations, dtype=np.float32)
    consts = _consts()
    maps = []
    for n in range(N):
        FA, FB = _features(images[n])
        segy = np.ascontiguousarray(
            segmentations[n].transpose(1, 0, 2).reshape(128, K * 128)
        ).astype(BF)  # [y, (k,x)]
        maps.append(dict(fa=FA, fb=FB, seg=segy, **consts))
    return maps


def _batches():
    """Work-item schedule: list of batches; each batch is a list of items
    (g, b, width, off) sharing one [128,BW] PSUM tile / one Exp. Diagonal
    512-blocks are emitted at 128 granularity packed [512,384,128,256]."""
    batches = []
    cur, off = [], 0
    for g in range(NG - 1, -1, -1):
        if FINE_DIAG:
            # diag batch: chunks 4g+3 (w512), 4g+2 (384), 4g (128), 4g+1 (256)
            batches.append([
                (g, 4 * g + 3, 512, 0),
                (g, 4 * g + 2, 384, 512),
                (g, 4 * g + 0, 128, 896),
                (g, 4 * g + 1, 256, 1024),
            ])
            lastb = 4 * g + 3
        else:
            lastb = 4 * g - 1
        for b in range(NB - 1, lastb, -1):  # full-width chunks, descending
            if g == 0 and b in DVE_CHUNKS:
                continue  # exp'd on DVE via the squaring chain
            cur.append((g, b, 512, off))
            off += 512
            if off == BW:
                batches.append(cur)
                cur, off = [], 0
    if cur:
        batches.append(cur)
    return batches


def _acc_writers(items):
    """Per group: ordered acc-matmul descriptors (g, b, scaled, lo, hi) in
    emission order, with start/stop flags per 128-col region computed by a
    forward (fresh) and backward (last-writer) pass."""
    per_g = {g: [] for g in range(NG)}
    for g, b, w, off in items:
        i = b - 4 * g
        if 0 <= i < 4 and FINE_DIAG:  # diag chunk: sub-diag (x2) + diag blk (x1)
            if i > 0:
                per_g[g].append([g, b, True, 0, i * 128, off])
            per_g[g].append([g, b, False, i * 128, (i + 1) * 128, off])
        elif 0 <= i < 4:  # coarse: whole diag chunk at x1
            per_g[g].append([g, b, False, 0, 512, off])
        else:
            per_g[g].append([g, b, True, 0, 512, off])
    # PSUM accumulation groups are per 2KB bank: start=True marks the WHOLE
    # bank pending-zero (first touch of each byte overwrites, later touches
    # accumulate). So: exactly one start (first matmul of the group's bank)
    # and one stop (last matmul).
    flags = {}
    for g, lst in per_g.items():
        n = len(lst)
        flags[g] = [
            (tuple(d), i == 0, i == n - 1) for i, d in enumerate(lst)
        ]
    return flags


def _build():
    nc = bacc.Bacc()
    fa_d = nc.dram_tensor("fa", [FR, P], BF16, kind="ExternalInput")
    fb_d = nc.dram_tensor("fb", [FR, P], BF16, kind="ExternalInput")
    seg_d = nc.dram_tensor("seg", [128, K * 128], BF16, kind="ExternalInput")
    rt_d = nc.dram_tensor("rt", [128, 64], BF16, kind="ExternalInput")
    idf_d = nc.dram_tensor("idf", [K, K], F32, kind="ExternalInput")
    out_d = nc.dram_tensor("out", [K, NG], F32, kind="ExternalOutput")

    batches = _batches()
    nbatch = len(batches)
    # map (g,b) -> batch index for acc scheduling; group -> last batch idx
    item_batch = {}
    glast = {}
    for bi, items in enumerate(batches):
        for g, b, w, off in items:
            item_batch[(g, b)] = bi
            glast[g] = bi
    acc_flags = _acc_writers([it for bt in batches for it in bt])
    # per-batch acc descriptors (in group emission order)
    bat_accs = [[] for _ in range(nbatch)]
    for g in range(NG - 1, -1, -1):
        for (gg, b, scaled, lo, hi, off), st, sp in acc_flags[g]:
            bat_accs[item_batch[(gg, b)]].append((gg, b, scaled, lo, hi, off, st, sp))

    with tile.TileContext(nc) as tc:
        with (
            tc.tile_pool(name="pp", bufs=1) as pp,
            tc.tile_pool(name="gq", bufs=2, space="PSUM") as gq,
            tc.tile_pool(name="aq", bufs=2, space="PSUM") as aq,
            tc.tile_pool(name="ep", bufs=9) as ep,
            tc.tile_pool(name="dp", bufs=2) as dp,
        ):
            FAs = pp.tile([FR, P], BF16, tag="fa", name="FAs")
            FBs = pp.tile([FR, P], BF16, tag="fb", name="FBs")
            seg_s = pp.tile([128, K * 128], BF16, tag="seg")
            rt_s = pp.tile([128, 64], BF16, tag="rt")
            idf_s = pp.tile([K, K], F32, tag="idf")
            At = pp.tile([128, K * 64], BF16, tag="At")
            Srow = [pp.tile([K, 512], F32, tag=f"sr{y}", name=f"sr{y}") for y in range(NG)]
            STt1 = [pp.tile([128, 8 * K], BF16, tag=f"t1{i}", name=f"t1{i}") for i in range(4)]
            STt2 = [pp.tile([128, 8 * K], BF16, tag=f"t2{i}", name=f"t2{i}") for i in range(4)]
            partials = pp.tile([K, NG], F32, tag="partials")

            qS = nc.sync.dma_start
            qP = nc.gpsimd.dma_start

            # ---- input loads. FA/FB head slices land first (smallest head =
            # batch 0's columns) so the G pipeline starts ~2us in; the rest
            # follows behind the seg halves. ----
            H1, H2 = 3584, 2560
            qS(FAs[:, H1:], fa_d[:, H1:])
            qP(FBs[:, H1:], fb_d[:, H1:])
            qS(FAs[:, H2:H1], fa_d[:, H2:H1])
            qP(FBs[:, H2:H1], fb_d[:, H2:H1])
            qS(rt_s[:], rt_d[:])
            qP(seg_s[:, : 8 * 128], seg_d[:, : 8 * 128])
            qS(seg_s[:, 8 * 128 : 16 * 128], seg_d[:, 8 * 128 : 16 * 128])
            qP(idf_s[:], idf_d[:])
            qS(seg_s[:, 16 * 128 :], seg_d[:, 16 * 128 :])
            qP(FBs[:, :H2], fb_d[:, :H2])
            qS(FAs[:, :H2], fa_d[:, :H2])

            # ---- PE p-state warmup: keep the PE busy before the first G
            # batch so the early matmuls run at mid/full clock ----
            wsb = pp.tile([2, 256], BF16, tag="warm")
            nc.vector.memset(wsb[:], 1.0)
            for wi in range(8):
                wps = aq.tile([128, 512], F32, tag="a", name=f"warm{wi}")
                nc.tensor.matmul(
                    wps[0:1, 0:256], wsb[:, 0:1], wsb[:], start=True, stop=True
                )

            at3 = At[:, :].rearrange("x (k y) -> x k y", k=K, y=64)

            def at_block(k0, k1):
                aps = aq.tile([128, 512], F32, tag="a", name=f"at{k0}")
                for k in range(k0, k1):
                    nc.tensor.matmul(
                        aps[:, (k - k0) * 64 : (k - k0 + 1) * 64],
                        seg_s[:, k * 128 : (k + 1) * 128],
                        rt_s[:],
                        start=True, stop=True,
                    )
                nc.vector.tensor_copy(At[:, k0 * 64 : k1 * 64], aps[:, : (k1 - k0) * 64])

            def srow_stage(yb):
                sps = aq.tile([128, 512], F32, tag="a", name=f"sr{yb}")
                for yl in range(8):
                    yp = yb * 8 + yl
                    nc.tensor.matmul(
                        sps[0:K, yl * 64 : (yl + 1) * 64],
                        at3[:, :, yp], rt_s[:],
                        start=True, stop=True,
                    )
                nc.vector.tensor_copy(Srow[yb][:], sps[0:K, :])

            def stt_stage(bi):
                tps = aq.tile([128, 512], F32, tag="a", name=f"st{bi}")
                for j in range(8):
                    b = 8 * bi + j
                    yb, rest = divmod(b * 128, 512)
                    nc.tensor.transpose(
                        tps[:, j * K : (j + 1) * K],
                        Srow[yb][:, rest : rest + 128],
                        idf_s[:],
                    )
                nc.vector.tensor_copy(STt1[bi][:], tps[:, : 8 * K])
                nc.vector.tensor_scalar_mul(STt2[bi][:], tps[:, : 8 * K], 2.0)

            # seg-pipeline emission steps: all scratch-tile (aq) requests must
            # precede the first acc-tile request (batch ACC_LAG) or the
            # 2-buffer rotation deadlocks against a live accumulator.
            seg_steps = {
                1: [lambda: at_block(0, 8)],
                2: [lambda: at_block(8, 16)],
                3: [lambda: at_block(16, 21), lambda: srow_stage(7)],
                4: [lambda: srow_stage(6), lambda: stt_stage(3)],
                5: [lambda: srow_stage(5), lambda: srow_stage(4)],
                6: [lambda: stt_stage(2), lambda: srow_stage(3)],
                7: [lambda: srow_stage(2), lambda: stt_stage(1),
                    lambda: srow_stage(1), lambda: srow_stage(0),
                    lambda: stt_stage(0)],
            }
            ACC_LAG = 8  # batches between exp and its acc matmuls

            acc_tiles = {}

            def emit_accs(bi, ets):
                for g, b, scaled, lo, hi, off, st, sp in bat_accs[bi]:
                    if g not in acc_tiles:
                        acc_tiles[g] = aq.tile([128, 512], F32, tag="a", name=f"acc{g}")
                    stt = (STt2 if scaled else STt1)[b // 8]
                    nc.tensor.matmul(
                        acc_tiles[g][0:K, lo:hi],
                        stt[:, (b % 8) * K : (b % 8 + 1) * K],
                        ets[bi][:, off + lo : off + hi],
                        start=st, stop=sp,
                        skip_group_check=True,
                    )

            def emit_dots(bi):
                done = {g for g in range(NG) if glast[g] == bi}
                for g in sorted(done, reverse=True):
                    dsc = dp.tile([K, 512], BF16, tag="d", name=f"dsc{g}")
                    if USE_TTR:
                        nc.vector.tensor_tensor_reduce(
                            dsc[:], acc_tiles[g][0:K, :], Srow[g][:],
                            1.0, 0.0, ALU.mult, ALU.add,
                            partials[:, g : g + 1],
                        )
                    else:
                        nc.vector.tensor_mul(dsc[:], acc_tiles[g][0:K, :], Srow[g][:])
                        nc.vector.tensor_reduce(
                            partials[:, g : g + 1], dsc[:],
                            mybir.AxisListType.X, ALU.add,
                        )

            # DVE exp chain state: E = (1 + G/2^12)^(2^12) in f32 ping-pong
            # buffers, one squaring every other batch (keeps the DVE queue
            # drained so dots/copies aren't delayed).
            ua = pp.tile([128, BW], F32, tag="ua")
            ub = pp.tile([128, BW], F32, tag="ub")
            etsp = pp.tile([128, BW], BF16, tag="etsp")
            NSQ = 12
            sq_sched = {11 + 2 * k: k for k in range(NSQ)}  # batch -> square idx

            def special_g():
                gts = gq.tile([128, BW], F32, tag="g", name="gspec")
                for j, b in enumerate(DVE_CHUNKS):
                    nc.tensor.matmul(
                        gts[:, j * 512 : (j + 1) * 512],
                        FAs[:, b * 128 : (b + 1) * 128],
                        FBs[:, 0:512],
                        start=True, stop=True,
                    )
                nc.vector.tensor_scalar(
                    ua[:], gts[:], float(2.0 ** -NSQ), 1.0, ALU.mult, ALU.add
                )

            def special_sq(k):
                src = ua if k % 2 == 0 else ub
                dst = ub if k % 2 == 0 else ua
                if k == NSQ - 1:
                    nc.vector.tensor_tensor(etsp[:], src[:], src[:], ALU.mult)
                else:
                    nc.vector.tensor_tensor(dst[:], src[:], src[:], ALU.mult)

            def special_accs():
                for j, b in enumerate(DVE_CHUNKS):
                    nc.tensor.matmul(
                        acc_tiles[0][0:K, 0:512],
                        STt2[b // 8][:, (b % 8) * K : (b % 8 + 1) * K],
                        etsp[:, j * 512 : (j + 1) * 512],
                        start=False, stop=False,
                        skip_group_check=True,
                    )

            ets = {}
            for bi, items in enumerate(batches):
                gt = gq.tile([128, BW], F32, tag="g", name=f"g{bi}")
                width = max(off + w for _, _, w, off in items)
                for g, b, w, off in items:
                    nc.tensor.matmul(
                        gt[:, off : off + w],
                        FAs[:, b * 128 : (b + 1) * 128],
                        FBs[:, 512 * g : 512 * g + w],
                        start=True, stop=True,
                    )
                et = ep.tile([128, BW], BF16, tag="e", name=f"e{bi}")
                ets[bi] = et
                nc.scalar.activation(et[:, :width], gt[:, :width], AF.Exp)
                for fn in seg_steps.get(bi, ()):
                    fn()
                if bi == 9:
                    special_g()
                if bi in sq_sched:
                    special_sq(sq_sched[bi])
                if bi >= ACC_LAG:
                    emit_accs(bi - ACC_LAG, ets)
                    emit_dots(bi - ACC_LAG)
                if bi == nbatch - 1:
                    special_accs()
            for bi in range(max(0, nbatch - ACC_LAG), nbatch):
                emit_accs(bi, ets)
                emit_dots(bi)

            # ---- tail: DMA the per-group partials; all summing on host ----
            nc.sync.dma_start(out_d[:], partials[:])

    nc.finalize()
    return nc


_CACHE = {}


def _get_nc():
    if "nc" not in _CACHE:
        _CACHE["nc"] = _build()
    return _CACHE["nc"]


def kernel(images: np.ndarray, segmentations: np.ndarray) -> np.ndarray:
    images = np.ascontiguousarray(np.asarray(images, dtype=np.float32))
    segmentations = np.ascontiguousarray(np.asarray(segmentations, dtype=np.float32))
    assert images.shape == (N, C, H, W) and segmentations.shape == (N, K, 128, 128)
    nc = _get_nc()
    in_maps = _prep(images, segmentations)
    res = run_bass_kernel_spmd(nc, in_maps, list(range(N)))
    total = sum(float(res.results[n]["out"].sum()) for n in range(N))
    return np.array([-WEIGHT / N * total], dtype=np.float32)


if __name__ == "__main__":
    rng = np.random.RandomState(0)
    img = rng.rand(N, C, H, W).astype(np.float32) * 255.0
    seg = rng.rand(N, K, 128, 128).astype(np.float32)
    print(kernel(img, seg))
